# revision 6
# baseline (speedup 1.0000x reference)
"""Trainium2 Bass kernel for nn_ExpertsFeedForward (MoE expert-choice routing).

Sharding: owner-parallel over tokens. Each of the 8 cores owns a 2048-token
slice of the flattened [16384, 1024] input and produces that slice of the
output. FF expert weights are replicated (streamed from HBM, bf16); each core
computes the shared expert for its tokens plus every FF expert's contribution
to its tokens (dma_gather -> FFN -> score scale -> CCE dma_scatter_add), with
the constant 'jump' expert and all biases folded in as rank-1 matmuls.
"""

import numpy as np
import ml_dtypes

import concourse.bass as bass
import concourse.mybir as mybir
import concourse.bacc as bacc
import concourse.tile as tile
from concourse.bass_utils import run_bass_kernel_spmd
from concourse.library_config import mlp as mlp_lib

BF16 = mybir.dt.bfloat16
F32 = mybir.dt.float32
I16 = mybir.dt.int16
U32 = mybir.dt.uint32

NC = 8            # cores
D = 1024          # d_model
H = 4096          # d_ff
FF = 7            # matmul experts
NTOK = 16384      # total tokens
TOK = NTOK // NC  # tokens per core (2048)
CAP = 1638        # expert capacity (global)
BUCKET = 256      # per-(core, expert) selected-token capacity (padded)
KD = D // 128     # 8  contraction tiles over d
MH = H // 128     # 32 tiles over h
ND2 = 2           # 512-wide n chunks over D
CH = 256          # shared-FFN token chunk
NCH = TOK // CH   # 8 chunks
TT = TOK // 128   # 16 row tiles
GELU = mybir.ActivationFunctionType.Gelu_apprx_tanh


def build_program():
    nc = bacc.Bacc("TRN2", target_bir_lowering=False, debug=False, num_devices=NC)

    # ---- per-core external inputs ----
    xT = nc.dram_tensor("xT", [KD, 128, NCH, CH], BF16, kind="ExternalInput")
    xrows = nc.dram_tensor("xrows", [TOK, D], BF16, kind="ExternalInput")
    wk = nc.dram_tensor("wk", [FF, KD, MH, 128, 128], BF16, kind="ExternalInput")
    wv = nc.dram_tensor("wv", [FF, MH, ND2, 128, 512], BF16, kind="ExternalInput")
    swk = nc.dram_tensor("swk", [KD, MH, 128, 128], BF16, kind="ExternalInput")
    swv = nc.dram_tensor("swv", [MH, ND2, 128, 512], BF16, kind="ExternalInput")
    bk = nc.dram_tensor("bk", [FF, MH, 128], F32, kind="ExternalInput")
    sbk = nc.dram_tensor("sbk", [MH, 128], F32, kind="ExternalInput")
    bvr = nc.dram_tensor("bvr", [FF, 1, D], BF16, kind="ExternalInput")
    sbvr = nc.dram_tensor("sbvr", [1, D], BF16, kind="ExternalInput")
    jumpv = nc.dram_tensor("jumpv", [1, D], BF16, kind="ExternalInput")
    onesr = nc.dram_tensor("onesr", [1, 128], BF16, kind="ExternalInput")
    idxg = nc.dram_tensor("idxg", [FF, 128, BUCKET // 16], I16, kind="ExternalInput")
    idxs = nc.dram_tensor("idxs", [FF, 128, BUCKET // 16], I16, kind="ExternalInput")
    scores = nc.dram_tensor("scores", [FF, BUCKET // 128, 128], F32,
                            kind="ExternalInput")
    ms7 = nc.dram_tensor("ms7", [1, TOK], BF16, kind="ExternalInput")

    out = nc.dram_tensor("out", [TOK, D], F32, kind="ExternalOutput")

    # ---- internal DRAM scratch ----
    eout = nc.dram_tensor("eout", [TOK + 128, D], BF16)  # + dummy pad rows
    sout = nc.dram_tensor("sout", [TOK, D], F32)    # shared+jump dense

    with tile.TileContext(nc) as tc:
        with (
            tc.tile_pool(name="const", bufs=1) as cpool,
            tc.tile_pool(name="psum1", bufs=2, space="PSUM") as pp1,
            tc.tile_pool(name="psum2", bufs=2, space="PSUM") as pp2,
        ):
            nc.gpsimd.load_library(mlp_lib)

            def gelu_tanh(out_ap, ps_ap, bias_ap, tmp_pool, n, tag):
                """out = gelu_tanh(ps + bias); ps is PSUM f32 [128, n]."""
                xb = tmp_pool.tile([128, n], F32, tag=tag + "x")
                s = tmp_pool.tile([128, n], F32, tag=tag + "s")
                nc.vector.tensor_scalar_add(xb[:], ps_ap, bias_ap)
                nc.vector.tensor_tensor(s[:], xb[:], xb[:], op=mybir.AluOpType.mult)
                nc.vector.tensor_scalar(s[:], s[:], 0.044715, 1.0,
                                        op0=mybir.AluOpType.mult,
                                        op1=mybir.AluOpType.add)
                nc.vector.tensor_tensor(s[:], s[:], xb[:], op=mybir.AluOpType.mult)
                nc.scalar.activation(s[:], s[:], mybir.ActivationFunctionType.Tanh,
                                     scale=0.7978845608028654)
                nc.vector.tensor_scalar(s[:], s[:], 0.5, 0.5,
                                        op0=mybir.AluOpType.mult,
                                        op1=mybir.AluOpType.add)
                nc.vector.tensor_tensor(out_ap, s[:], xb[:],
                                        op=mybir.AluOpType.mult)

            # --- constants resident in SBUF for the whole kernel ---
            ones_sb = cpool.tile([1, 128], BF16, tag="ones")
            nc.sync.dma_start(out=ones_sb[:], in_=onesr[:])
            jump_sb = cpool.tile([1, D], BF16, tag="jump")
            nc.sync.dma_start(out=jump_sb[:], in_=jumpv[:])
            sbv_sb = cpool.tile([1, D], BF16, tag="sbv")
            nc.sync.dma_start(out=sbv_sb[:], in_=sbvr[:])
            bv_sb = cpool.tile([1, FF * D], BF16, tag="bv")
            nc.sync.dma_start(out=bv_sb[:], in_=bvr[:].rearrange("e o d -> o (e d)"))
            ms7_sb = cpool.tile([1, TOK], BF16, tag="ms7")
            nc.sync.dma_start(out=ms7_sb[:], in_=ms7[:])
            zero_sb = cpool.tile([128, D], BF16, tag="zero")
            nc.vector.memset(zero_sb[:], 0.0)
            for t in range(TT + 1):
                nc.sync.dma_start(out=eout[t * 128:(t + 1) * 128, :], in_=zero_sb[:])

            # ================= phase 1: FF experts =================
            with (
                tc.tile_pool(name="ep_io", bufs=2) as eio,
                tc.tile_pool(name="ep_w", bufs=3) as ewp,
                tc.tile_pool(name="ep_acts", bufs=1) as eap,
            ):
                for e in range(FF):
                    idxg_sb = eio.tile([128, BUCKET // 16], I16, tag="idxg")
                    nc.sync.dma_start(out=idxg_sb[:], in_=idxg[e])
                    idxs_sb = eio.tile([128, BUCKET // 16], I16, tag="idxs")
                    nc.sync.dma_start(out=idxs_sb[:], in_=idxs[e])
                    xs = eap.tile([128, KD, BUCKET], BF16, tag="xs")
                    nc.gpsimd.dma_gather(
                        xs[:], xrows[:], idxg_sb[:], BUCKET, BUCKET, D,
                        transpose=True,
                    )
                    # expert Wv resident for this expert (read once)
                    wv_sb = eap.tile([128, MH, ND2, 512], BF16, tag="wv_e")
                    for m in range(MH):
                        for n in range(ND2):
                            nc.sync.dma_start(out=wv_sb[:, m, n, :], in_=wv[e, m, n])
                    bk_sb = eio.tile([128, MH], F32, tag="bk")
                    nc.sync.dma_start(out=bk_sb[:], in_=bk[e].rearrange("a p -> p a"))
                    sc_sb = eio.tile([128, BUCKET // 128], F32, tag="sc")
                    nc.sync.dma_start(
                        out=sc_sb[:], in_=scores[e].rearrange("a p -> p a"))

                    hT = eap.tile([128, MH, BUCKET], BF16, tag="hT_e")
                    for m in range(MH):
                        ps = pp1.tile([128, BUCKET], F32, tag="ps1")
                        for k in range(KD):
                            wt = ewp.tile([128, 128], BF16, tag="w1")
                            nc.sync.dma_start(out=wt[:], in_=wk[e, k, m])
                            nc.tensor.matmul(ps[:], wt[:], xs[:, k, :],
                                             start=(k == 0), stop=(k == KD - 1))
                        gelu_tanh(hT[:, m, :], ps[:], bk_sb[:, m:m + 1],
                                  eio, BUCKET, "ge")
                    ysb = eap.tile([128, BUCKET // 128, D], BF16, tag="ysb")
                    for tt in range(BUCKET // 128):
                        for n in range(ND2):
                            ps2 = pp2.tile([128, 512], F32, tag="ps2")
                            for m in range(MH):
                                nc.tensor.matmul(
                                    ps2[:], hT[:, m, tt * 128:(tt + 1) * 128],
                                    wv_sb[:, m, n, :], start=(m == 0), stop=False)
                            nc.tensor.matmul(
                                ps2[:], ones_sb[:],
                                bv_sb[:, e * D + n * 512:e * D + (n + 1) * 512],
                                start=False, stop=True)
                            nc.vector.tensor_scalar_mul(
                                ysb[:, tt, n * 512:(n + 1) * 512], ps2[:],
                                sc_sb[:, tt:tt + 1])
                    nc.gpsimd.dma_scatter_add(
                        eout[:], ysb[:], idxs_sb[:], BUCKET, BUCKET, D,
                        queue_num=0)

            # ================= phase 2: shared expert =================
            with (
                tc.tile_pool(name="sp_io", bufs=2) as sio,
                tc.tile_pool(name="sp_w", bufs=1) as swp,
                tc.tile_pool(name="sp_acts", bufs=2) as sap,
            ):
                swk_sb = swp.tile([128, KD, MH, 128], BF16, tag="swk")
                for k in range(KD):
                    for m in range(MH):
                        nc.sync.dma_start(out=swk_sb[:, k, m, :], in_=swk[k, m])
                swv_sb = swp.tile([128, MH, ND2, 512], BF16, tag="swv")
                for m in range(MH):
                    for n in range(ND2):
                        nc.sync.dma_start(out=swv_sb[:, m, n, :], in_=swv[m, n])
                sbk_sb = swp.tile([128, MH], F32, tag="sbk")
                nc.sync.dma_start(out=sbk_sb[:], in_=sbk[:].rearrange("a p -> p a"))

                for c in range(NCH):
                    xc = sio.tile([128, KD, CH], BF16, tag="xc")
                    for k in range(KD):
                        nc.sync.dma_start(out=xc[:, k, :], in_=xT[k, :, c, :])
                    hT = sap.tile([128, MH, CH], BF16, tag="hT_s")
                    for m in range(MH):
                        ps = pp1.tile([128, CH], F32, tag="ps1")
                        for k in range(KD):
                            nc.tensor.matmul(ps[:], swk_sb[:, k, m, :], xc[:, k, :],
                                             start=(k == 0), stop=(k == KD - 1))
                        gelu_tanh(hT[:, m, :], ps[:], sbk_sb[:, m:m + 1],
                                  sio, CH, "gs")
                    for tt in range(CH // 128):
                        g0 = c * CH + tt * 128
                        so = sap.tile([128, D], F32, tag="so")
                        for n in range(ND2):
                            ps2 = pp2.tile([128, 512], F32, tag="ps2")
                            for m in range(MH):
                                nc.tensor.matmul(
                                    ps2[:], hT[:, m, tt * 128:(tt + 1) * 128],
                                    swv_sb[:, m, n, :], start=(m == 0), stop=False)
                            nc.tensor.matmul(
                                ps2[:], ones_sb[:], sbv_sb[:, n * 512:(n + 1) * 512],
                                start=False, stop=False)
                            nc.tensor.matmul(
                                ps2[:], ms7_sb[:, g0:g0 + 128],
                                jump_sb[:, n * 512:(n + 1) * 512],
                                start=False, stop=True)
                            nc.vector.tensor_copy(so[:, n * 512:(n + 1) * 512], ps2[:])
                        nc.sync.dma_start(out=sout[g0:g0 + 128, :], in_=so[:])

            # ================= phase 3: combine =================
            with tc.tile_pool(name="fp", bufs=3) as fp:
                for t in range(TT):
                    a = fp.tile([128, D], F32, tag="fa")
                    b = fp.tile([128, D], BF16, tag="fb")
                    bf = fp.tile([128, D], F32, tag="fbf")
                    nc.sync.dma_start(out=a[:], in_=sout[t * 128:(t + 1) * 128, :])
                    nc.sync.dma_start(out=b[:], in_=eout[t * 128:(t + 1) * 128, :])
                    nc.vector.tensor_copy(bf[:], b[:])
                    nc.vector.tensor_add(a[:], a[:], bf[:])
                    nc.sync.dma_start(out=out[t * 128:(t + 1) * 128, :], in_=a[:])

    nc.compile()
    return nc


def _bf(a):
    return np.ascontiguousarray(a.astype(ml_dtypes.bfloat16))


def host_route(x_flat, gate_W, gate_b, temperature):
    """Replicates the reference router + expert-choice top-k in numpy."""
    logits = x_flat.astype(np.float32) @ gate_W + gate_b
    t = max(float(np.asarray(temperature).reshape(-1)[0]), 0.1)
    z = logits / t
    z = z - z.max(axis=1, keepdims=True)
    p = np.exp(z)
    p = p / p.sum(axis=1, keepdims=True)
    order = np.argsort(-p, axis=0, kind="stable")
    sel = order[:CAP]  # [CAP, 8]
    return p, sel


def prepare_in_maps(inputs):
    x = np.asarray(inputs["x"], dtype=np.float32).reshape(NTOK, D)
    p, sel = host_route(
        x, np.asarray(inputs["gate_W"], np.float32),
        np.asarray(inputs["gate_b"], np.float32),
        np.asarray(inputs["temperature"], np.float32),
    )

    wk_t = _bf(np.asarray(inputs["Wk"], np.float32)
               .reshape(FF, KD, 128, MH, 128).transpose(0, 1, 3, 2, 4))
    wv_t = _bf(np.asarray(inputs["Wv"], np.float32)
               .reshape(FF, MH, 128, ND2, 512).transpose(0, 1, 3, 2, 4))
    swk_t = _bf(np.asarray(inputs["sWk"], np.float32)
                .reshape(KD, 128, MH, 128).transpose(0, 2, 1, 3))
    swv_t = _bf(np.asarray(inputs["sWv"], np.float32)
                .reshape(MH, 128, ND2, 512).transpose(0, 2, 1, 3))
    bk_t = np.ascontiguousarray(
        np.asarray(inputs["bk"], np.float32).reshape(FF, MH, 128))
    sbk_t = np.ascontiguousarray(
        np.asarray(inputs["sbk"], np.float32).reshape(MH, 128))
    bvr = _bf(np.asarray(inputs["bv"], np.float32).reshape(FF, 1, D))
    sbvr = _bf(np.asarray(inputs["sbv"], np.float32).reshape(1, D))
    jumpv = _bf(np.asarray(inputs["jump"], np.float32).reshape(1, D))
    onesr = _bf(np.ones((1, 128), np.float32))

    in_maps = []
    for c in range(NC):
        lo, hi = c * TOK, (c + 1) * TOK
        xs = x[lo:hi]
        xT_c = _bf(xs.T.reshape(KD, 128, NCH, CH))
        idxg_c = np.zeros((FF, BUCKET), np.int16)
        idxs_c = np.zeros((FF, BUCKET), np.int16)
        sc_c = np.zeros((FF, BUCKET), np.float32)
        for e in range(FF):
            g = np.sort(sel[:, e][(sel[:, e] >= lo) & (sel[:, e] < hi)])
            n = len(g)
            assert 0 < n <= BUCKET, f"bucket count {n} out of range"
            idxg_c[e, :n] = (g - lo).astype(np.int16)
            idxs_c[e, :n] = (g - lo).astype(np.int16)
            idxs_c[e, n:] = TOK + np.arange(BUCKET - n, dtype=np.int16) % 128
            sc_c[e, :n] = p[g, e]
        # wrap indices [r] -> [r%16, r//16], tiled to 128 partitions
        def wrap(a):
            w = np.ascontiguousarray(a.reshape(FF, BUCKET // 16, 16).transpose(0, 2, 1))
            return np.tile(w, (1, 8, 1))
        idxg_w = wrap(idxg_c)
        idxs_w = wrap(idxs_c)
        sc_pt = np.ascontiguousarray(sc_c.reshape(FF, BUCKET // 128, 128))
        m7 = sel[:, FF][(sel[:, FF] >= lo) & (sel[:, FF] < hi)]
        ms7_c = np.zeros(TOK, np.float32)
        ms7_c[m7 - lo] = p[m7, FF]
        in_maps.append({
            "xT": xT_c,
            "xrows": _bf(xs),
            "wk": wk_t, "wv": wv_t, "swk": swk_t, "swv": swv_t,
            "bk": bk_t, "sbk": sbk_t, "bvr": bvr, "sbvr": sbvr,
            "jumpv": jumpv, "onesr": onesr,
            "idxg": idxg_w, "idxs": idxs_w, "scores": sc_pt,
            "ms7": _bf(ms7_c.reshape(1, TOK)),
        })
    return in_maps


_CACHED = None


def kernel(**inputs):
    global _CACHED
    if _CACHED is None:
        _CACHED = build_program()
    nc = _CACHED
    in_maps = prepare_in_maps(inputs)
    res = run_bass_kernel_spmd(nc, in_maps, list(range(NC)))
    out = np.concatenate([res.results[c]["out"] for c in range(NC)], axis=0)
    return out.reshape(8, 2048, 1024).astype(np.float32)


if __name__ == "__main__":
    d = np.load("/root/problem/ref_inputs.npz")
    exp = np.load("/root/problem/ref_out.npy")
    got = kernel(**{k: d[k] for k in d.files})
    err = np.abs(got - exp)
    print("absmax rel:", err.max() / np.abs(exp).max())
    print("rms rel:", np.sqrt((err ** 2).mean()) / exp.std())
